# revision 21
# baseline (speedup 1.0000x reference)
"""CenterPooling (CornerNet) Trainium2 kernel — 8 NeuronCores.

Sharding: 8 cores = 4 batches x 2 H-halves.  Each core gets a host-padded
input slab [256, 70, 130] (3 halo rows each side, zero W-pad columns).

Key algebraic simplifications:
 - cummax(reverse) then cummax(forward) along an axis == global max along
   that axis, broadcast.  So the up branch only needs per-row maxes over W
   ([C, H]) and the down branch per-column maxes over H ([C, W]).
 - BN (eval mode) folds into conv weights/bias on the host; BN scale > 0 so
   max-reduction commutes with the affine+ReLU epilogue.
 - The merge conv's input is rank-structured: updown[c,h,w] = u[c,h] + d[c,w],
   so the 3x3 merge conv SEPARATES into tiny 1-D convs: an h-conv of u
   ([C, H] -> A(o,h), with 3 w-boundary classes of kx-summed weights) plus a
   w-conv of d ([C, W] -> B(o,w), with h-boundary corrections -Bk0/-Bk2 at
   the global top/bottom rows, applied data-driven via per-row selector
   vectors).  This removes the 18 big matmuls per merge block entirely.
 - Down-branch col-max needs a cross-half combine: pairwise AllReduce-max of
   a tiny [256, 128] tile.
 - H-pad semantics at the global top/bottom are handled data-driven (SPMD
   uniform program): a validity mask zeroes invalid u rows, and the per-row
   ACT bias adds -1e30 to out-of-range relu1 rows so ReLU clamps them to
   the zero-pad value.

Convs run as 9 shifted bf16 matmuls per ci-tile accumulating into fp32 PSUM
(N=512 moving operand = 4 output rows x 128 cols).
"""

import sys

sys.path.insert(0, "/opt/trn_rl_repo")

import numpy as np
import ml_dtypes

import concourse.bacc as bacc
import concourse.tile as tile
import concourse.bass as bass
from concourse import mybir, bass_utils

BF16 = mybir.dt.bfloat16
F32 = mybir.dt.float32
NP_BF16 = ml_dtypes.bfloat16

N_CORES = 8
B, CIN, C, H, W = 4, 256, 256, 128, 128
G = 3            # halo rows on each side of the 64 owned rows
HS = 64 + 2 * G  # 70 slab rows
WP = W + 2       # 130 (zero-pad col on each side)
EPS = 1e-5
NEG = -1e30

RELU = mybir.ActivationFunctionType.Relu
AX_X = mybir.AxisListType.X
ALU = mybir.AluOpType

_CACHE: dict = {}


def _mm_group(nc, ps_ap, mms):
    n = len(mms)
    for k, (lhsT, rhs) in enumerate(mms):
        nc.tensor.matmul(ps_ap, lhsT, rhs, start=(k == 0), stop=(k == n - 1))


def _conv3_mms(wtile, src, s, nr, cot):
    """The 18 (ci,ky,kx) matmuls of a 3x3 conv block: output rows s..s+nr-1."""
    mms = []
    for cit in range(2):
        for ky in range(3):
            for kx in range(3):
                j = ((ky * 3 + kx) * 2 + cit) * 2 + cot
                mms.append((wtile[:, j, :], src[cit][:, s + ky - 1:s + ky - 1 + nr, kx:kx + W]))
    return mms


def _build():
    nc = bacc.Bacc("TRN2", target_bir_lowering=False, debug=False,
                   num_devices=N_CORES)

    x_d = nc.dram_tensor("x", [2, 128, HS, WP], BF16, kind="ExternalInput")
    wup_d = nc.dram_tensor("wup", [128, 36, 128], BF16, kind="ExternalInput")
    wdn_d = nc.dram_tensor("wdn", [128, 36, 128], BF16, kind="ExternalInput")
    wc2_d = nc.dram_tensor("wc2", [128, 36, 128], BF16, kind="ExternalInput")
    wc1_d = nc.dram_tensor("wc1", [128, 4, 128], BF16, kind="ExternalInput")
    wa_d = nc.dram_tensor("wa", [128, 36, 128], BF16, kind="ExternalInput")
    wb_d = nc.dram_tensor("wb", [128, 36, 128], BF16, kind="ExternalInput")
    bias_d = nc.dram_tensor("biases", [128, 8], F32, kind="ExternalInput")
    hv_d = nc.dram_tensor("hv", [128, HS], F32, kind="ExternalInput")
    pnegb_d = nc.dram_tensor("pnegb", [128, HS], F32, kind="ExternalInput")
    htop_d = nc.dram_tensor("htopneg", [128, HS], F32, kind="ExternalInput")
    hbot_d = nc.dram_tensor("hbotneg", [128, HS], F32, kind="ExternalInput")
    out_d = nc.dram_tensor("out", [2, 128, 64, W], F32, kind="ExternalOutput")

    with tile.TileContext(nc) as tc:
        with tc.tile_pool(name="const", bufs=1) as constp, \
             tc.tile_pool(name="acts", bufs=1) as actp, \
             tc.tile_pool(name="psum", bufs=6, space="PSUM") as psp, \
             tc.tile_pool(name="ostage", bufs=6) as osp, \
             tc.tile_pool(name="dram", bufs=1, space="DRAM") as dramp:

            wdn = constp.tile([128, 36, 128], BF16)
            # first two chunks smaller: the first MM group needs j=0/1 only
            for j0, jn in [(0, 4), (4, 5), (9, 9), (18, 9), (27, 9)]:
                nc.sync.dma_start(wdn[:, j0:j0 + jn, :], wdn_d.ap()[:, j0:j0 + jn, :])

            xs = []
            for cit in range(2):
                xt = actp.tile([128, HS, WP], BF16, name=f"xs{cit}")
                xs.append(xt)
            # interleave cit0/cit1 chunks so both tiles' early rows land first
            # first chunk split in two so the first conv block's rows [2,8)
            # arrive via parallel rings
            chunks = [(0, 4), (4, 4)] + [(r0, min(8, HS - r0)) for r0 in range(8, HS, 8)]
            for r0, rn in chunks:
                for cit in range(2):
                    nc.sync.dma_start(xs[cit][:, r0:r0 + rn, :], x_d.ap()[cit, :, r0:r0 + rn, :])

            wup = constp.tile([128, 36, 128], BF16)
            nc.sync.dma_start(wup[:, :, :], wup_d.ap())
            wc2 = constp.tile([128, 36, 128], BF16)
            nc.sync.dma_start(wc2[:, :, :], wc2_d.ap())
            wc1 = constp.tile([128, 4, 128], BF16)
            nc.sync.dma_start(wc1[:, :, :], wc1_d.ap())
            wa = constp.tile([128, 36, 128], BF16)
            nc.sync.dma_start(wa[:, :, :], wa_d.ap())
            wb = constp.tile([128, 36, 128], BF16)
            nc.sync.dma_start(wb[:, :, :], wb_d.ap())
            biases = constp.tile([128, 8], F32)
            nc.sync.dma_start(biases[:, :], bias_d.ap())
            hv = constp.tile([128, HS], F32)
            nc.sync.dma_start(hv[:, :], hv_d.ap())
            pnegb = constp.tile([128, HS], F32)
            nc.sync.dma_start(pnegb[:, :], pnegb_d.ap())
            htopneg = constp.tile([128, HS], F32)
            nc.sync.dma_start(htopneg[:, :], htop_d.ap())
            hbotneg = constp.tile([128, HS], F32)
            nc.sync.dma_start(hbotneg[:, :], hbot_d.ap())

            r1 = []
            for cit in range(2):
                t2 = actp.tile([128, HS, WP], BF16, name=f"r1{cit}")
                nc.vector.memset(t2[:, :, 0], 0.0)
                nc.vector.memset(t2[:, :, WP - 1], 0.0)
                r1.append(t2)

            uraw, ufin, dacc, dmax, dfin = [], [], [], [], []
            for cot in range(2):
                t = actp.tile([128, HS], F32, name=f"uraw{cot}")
                nc.vector.memset(t[:, :], 0.0)
                uraw.append(t)
                ufin.append(actp.tile([128, HS], F32, name=f"ufin{cot}"))
                t = actp.tile([128, W], F32, name=f"dacc{cot}")
                nc.vector.memset(t[:, :], -3e38)
                dacc.append(t)
                dmax.append(actp.tile([128, W], F32, name=f"dmax{cot}"))
                dfin.append(actp.tile([128, W], F32, name=f"dfin{cot}"))

            # ---- down branch: conv over the 64 owned rows, col-max over H ----
            for i in range(16):
                s = G + 4 * i
                for cot in range(2):
                    ps = psp.tile([128, 4, 128], F32, tag="ps", name="ps_dn", bufs=3)
                    _mm_group(nc, ps[:, :, :], _conv3_mms(wdn, xs, s, 4, cot))
                    for rr in range(4):
                        nc.vector.tensor_max(dacc[cot][:, :], dacc[cot][:, :], ps[:, rr, :])

            # pairwise (same-batch) AllReduce-max to get the global col-max
            cc_in = dramp.tile([256, W], F32)
            cc_out = dramp.tile([256, W], F32)
            for cot in range(2):
                nc.sync.dma_start(cc_in[cot * 128:(cot + 1) * 128, :], dacc[cot][:, :])
            nc.gpsimd.collective_compute(
                "AllReduce", ALU.max,
                replica_groups=[[0, 1], [2, 3], [4, 5], [6, 7]],
                ins=[cc_in.opt()], outs=[cc_out.opt()])
            for cot in range(2):
                nc.sync.dma_start(dmax[cot][:, :], cc_out[cot * 128:(cot + 1) * 128, :])
                nc.scalar.activation(dfin[cot][:, :], dmax[cot][:, :], RELU,
                                     bias=biases[:, 2 + cot:3 + cot], scale=1.0)

            # ---- up branch: conv over rows [1, 69), row-max over W ----
            for i in range(17):
                s = 1 + 4 * i
                for cot in range(2):
                    ps = psp.tile([128, 4, 128], F32, tag="ps", name="ps_up", bufs=3)
                    _mm_group(nc, ps[:, :, :], _conv3_mms(wup, xs, s, 4, cot))
                    nc.vector.reduce_max(uraw[cot][:, s:s + 4], ps[:, :, :], axis=AX_X)
            for cot in range(2):
                nc.scalar.activation(ufin[cot][:, :], uraw[cot][:, :], RELU,
                                     bias=biases[:, cot:cot + 1], scale=1.0)

            # ---- separable merge conv pieces ----
            # umask = u * hvalid (bf16), dpad = d with zero W-pad cols (bf16)
            umask, dpad = [], []
            for cit in range(2):
                t = actp.tile([128, HS], BF16, name=f"umask{cit}")
                nc.vector.tensor_mul(t[:, :], ufin[cit][:, :], hv[:, :])
                umask.append(t)
                t = actp.tile([128, WP], BF16, name=f"dpad{cit}")
                nc.vector.memset(t[:, :], 0.0)
                nc.vector.tensor_copy(t[:, 1:W + 1], dfin[cit][:, :])
                dpad.append(t)

            # A_cls(o,h): 1-D h-conv of umask with kx-summed merge weights.
            # cls 0=M (interior w), 1=L (w=0), 2=R (w=127).  Rows [2, 68).
            NA = 64 + 2
            asb = [[None, None, None], [None, None, None]]
            for cls in range(3):
                for cot in range(2):
                    psa = psp.tile([128, NA], F32, tag="psa", name="ps_a", bufs=2)
                    mms = []
                    for cit in range(2):
                        for ky in range(3):
                            j = ((cls * 3 + ky) * 2 + cit) * 2 + cot
                            mms.append((wa[:, j, :], umask[cit][:, 1 + ky:1 + ky + NA]))
                    _mm_group(nc, psa[:, :], mms)
                    t = actp.tile([128, NA], F32, name=f"asb{cls}{cot}")
                    nc.scalar.copy(t[:, :], psa[:, :])
                    asb[cot][cls] = t
            # afull = A_M + bias_pc1 + pneg (ACT bias per relu1 row);
            # afdL/afdR = A_L - A_M / A_R - A_M (w-edge fixups, pre-ReLU).
            afull, afdl, afdr = [], [], []
            for cot in range(2):
                t = actp.tile([128, HS], F32, name=f"afull{cot}")
                nc.vector.scalar_tensor_tensor(
                    t[:, 2:2 + NA], asb[cot][0][:, :], biases[:, 4 + cot:5 + cot],
                    pnegb[:, 2:2 + NA], op0=ALU.add, op1=ALU.add)
                afull.append(t)
                t = actp.tile([128, HS], F32, name=f"afdl{cot}")
                nc.vector.tensor_sub(t[:, 2:2 + NA], asb[cot][1][:, :], asb[cot][0][:, :])
                afdl.append(t)
                t = actp.tile([128, HS], F32, name=f"afdr{cot}")
                nc.vector.tensor_sub(t[:, 2:2 + NA], asb[cot][2][:, :], asb[cot][0][:, :])
                afdr.append(t)

            # B_var(o,w): 1-D w-conv of dpad with ky-summed merge weights.
            # var 0=M (all ky), 1=ky0 only, 2=ky2 only (boundary corrections).
            bt = [[None, None, None], [None, None, None]]
            for var in range(3):
                for cot in range(2):
                    psb = psp.tile([128, 128], F32, tag="psa", name="ps_b", bufs=2)
                    mms = []
                    for cit in range(2):
                        for kx in range(3):
                            j = ((var * 3 + kx) * 2 + cit) * 2 + cot
                            mms.append((wb[:, j, :], dpad[cit][:, kx:kx + W]))
                    _mm_group(nc, psb[:, :], mms)
                    t = actp.tile([128, 128], F32, name=f"bt{var}{cot}")
                    nc.scalar.copy(t[:, :], psb[:, :])
                    bt[cot][var] = t

            # ---- relu1 = relu(c1(x) + A + B + bias), assembled per block ----
            blocks = [(2 + 4 * i, 4) for i in range(16)] + [(66, 2)]
            for (s, nr) in blocks:
                for cot in range(2):
                    ps = psp.tile([128, nr, 128], F32, tag="ps", name="ps_p", bufs=3)
                    mms = []
                    for cit in range(2):
                        mms.append((wc1[:, cit * 2 + cot, :], xs[cit][:, s:s + nr, 1:W + 1]))
                    _mm_group(nc, ps[:, :, :], mms)
                    for r in range(nr):
                        sr = s + r
                        nc.vector.tensor_add(ps[:, r, :], ps[:, r, :], bt[cot][0][:, :])
                        # global top/bottom boundary corrections live at fixed
                        # slab rows (G and HS-G-1); the selector data zeroes
                        # them on the half where they don't apply.
                        if sr == G:
                            nc.vector.scalar_tensor_tensor(
                                ps[:, r, :], bt[cot][1][:, :], htopneg[:, sr:sr + 1],
                                ps[:, r, :], op0=ALU.mult, op1=ALU.add)
                        if sr == HS - G - 1:
                            nc.vector.scalar_tensor_tensor(
                                ps[:, r, :], bt[cot][2][:, :], hbotneg[:, sr:sr + 1],
                                ps[:, r, :], op0=ALU.mult, op1=ALU.add)
                    nc.vector.tensor_add(ps[:, :, 0], ps[:, :, 0], afdl[cot][:, s:s + nr])
                    nc.vector.tensor_add(ps[:, :, W - 1], ps[:, :, W - 1], afdr[cot][:, s:s + nr])
                    for r in range(nr):
                        sr = s + r
                        nc.scalar.activation(r1[cot][:, sr, 1:W + 1], ps[:, r, :], RELU,
                                             bias=afull[cot][:, sr:sr + 1], scale=1.0)

            # ---- output conv block ----
            for i in range(16):
                s = G + 4 * i
                for cot in range(2):
                    ps = psp.tile([128, 4, 128], F32, tag="ps2", name="ps_c2", bufs=3)
                    _mm_group(nc, ps[:, :, :], _conv3_mms(wc2, r1, s, 4, cot))
                    ot = osp.tile([128, 4, 128], F32, name="ot")
                    nc.scalar.activation(ot[:, :, :], ps[:, :, :], RELU,
                                         bias=biases[:, 6 + cot:7 + cot], scale=1.0)
                    if i >= 14:
                        # split the tail stores across rings so the last
                        # store's serial latency is halved
                        nc.sync.dma_start(out_d.ap()[cot, :, s - G:s - G + 2, :], ot[:, 0:2, :])
                        nc.sync.dma_start(out_d.ap()[cot, :, s - G + 2:s - G + 4, :], ot[:, 2:4, :])
                    else:
                        nc.sync.dma_start(out_d.ap()[cot, :, s - G:s - G + 4, :], ot[:, :, :])

    nc.compile()
    return nc


def _pack3(w):
    # [256o, 256i, 3, 3] -> [128ci, j, 128co], j = ((ky*3+kx)*2+cit)*2+cot
    a = w.reshape(2, 128, 2, 128, 3, 3).transpose(3, 4, 5, 2, 0, 1)
    return np.ascontiguousarray(a.reshape(128, 36, 128)).astype(NP_BF16)


def _pack1(w):
    # [256o, 256i, 1, 1] -> [128ci, j, 128co], j = cit*2+cot
    a = w[:, :, 0, 0].reshape(2, 128, 2, 128).transpose(3, 2, 0, 1)
    return np.ascontiguousarray(a.reshape(128, 4, 128)).astype(NP_BF16)


def _pack_sep(wk3):
    # wk3: [256o, 256i, 3] (kx- or ky-summed variants stacked on axis -1 by
    # caller as a dict) -> packs a [3var/cls, 3k, 256, 256] stack into
    # [128ci, j, 128co], j = ((v*3+k)*2+cit)*2+cot
    a = wk3.reshape(3, 3, 2, 128, 2, 128).transpose(5, 0, 1, 4, 2, 3)
    # dims now [ci, v, k, cit, cot, co]
    return np.ascontiguousarray(a.reshape(128, 36, 128)).astype(NP_BF16)


def _prep_in_maps(inputs):
    x = np.asarray(inputs["x"], dtype=np.float32)

    fw, fb = {}, {}
    for n in ["up", "down", "p", "c1", "c2"]:
        g = np.asarray(inputs[f"g_{n}"], np.float32)
        v = np.asarray(inputs[f"v_{n}"], np.float32)
        m = np.asarray(inputs[f"m_{n}"], np.float32)
        b = np.asarray(inputs[f"b_{n}"], np.float32)
        w = np.asarray(inputs[f"w_{n}"], np.float32)
        s = g / np.sqrt(v + EPS)
        fw[n] = w * s[:, None, None, None]
        fb[n] = b - m * s

    wp = fw["p"]
    wa_stack = np.stack([
        np.stack([wp[:, :, ky, :].sum(-1) for ky in range(3)]),            # M
        np.stack([wp[:, :, ky, 1:].sum(-1) for ky in range(3)]),           # L (w=0)
        np.stack([wp[:, :, ky, :2].sum(-1) for ky in range(3)]),           # R (w=127)
    ])
    wb_stack = np.stack([
        np.stack([wp[:, :, :, kx].sum(-1) for kx in range(3)]),            # M
        np.stack([wp[:, :, 0, kx] for kx in range(3)]),                    # ky=0
        np.stack([wp[:, :, 2, kx] for kx in range(3)]),                    # ky=2
    ])
    consts = {
        "wup": _pack3(fw["up"]),
        "wdn": _pack3(fw["down"]),
        "wc2": _pack3(fw["c2"]),
        "wc1": _pack1(fw["c1"]),
        "wa": _pack_sep(wa_stack),
        "wb": _pack_sep(wb_stack),
    }
    bias_np = np.zeros((128, 8), np.float32)
    for k, arr in enumerate([fb["up"], fb["down"], fb["p"] + fb["c1"], fb["c2"]]):
        m2 = arr.reshape(2, 128)
        bias_np[:, 2 * k] = m2[0]
        bias_np[:, 2 * k + 1] = m2[1]
    consts["biases"] = bias_np

    def _bcast(row):
        return np.ascontiguousarray(
            np.broadcast_to(row.astype(np.float32)[None, :], (128, HS)))

    in_maps = []
    for core in range(N_CORES):
        b_i, half = core // 2, core % 2
        slab = np.zeros((256, HS, WP), np.float32)
        if half == 0:
            slab[:, G:, 1:W + 1] = x[b_i][:, 0:HS - G, :]
            hv_row = (np.arange(HS) >= G)
            top_s, bot_s = G, None            # slab row of global row 0
        else:
            slab[:, :HS - G, 1:W + 1] = x[b_i][:, H - (HS - G):H, :]
            hv_row = (np.arange(HS) <= HS - G - 1)
            top_s, bot_s = None, HS - G - 1   # slab row of global row H-1
        xsl = np.ascontiguousarray(slab.reshape(2, 128, HS, WP)).astype(NP_BF16)
        pneg_row = np.where(hv_row, 0.0, NEG)
        htop_row = np.zeros(HS)
        if top_s is not None:
            htop_row[top_s] = -1.0
        hbot_row = np.zeros(HS)
        if bot_s is not None:
            hbot_row[bot_s] = -1.0
        in_maps.append({
            "x": xsl, "hv": _bcast(hv_row), "pnegb": _bcast(pneg_row),
            "htopneg": _bcast(htop_row), "hbotneg": _bcast(hbot_row), **consts})
    return in_maps


def _run(inputs, trace=False):
    # Build a fresh Bass program per call: re-executing an already-loaded
    # NEFF on these cores intermittently trips NRT_EXEC_UNIT_UNRECOVERABLE,
    # while a fresh build+load is reliable (neuronxcc cache keeps it fast).
    nc = _build()
    in_maps = _prep_in_maps(inputs)
    res = bass_utils.run_bass_kernel_spmd(
        nc, in_maps, core_ids=list(range(N_CORES)), trace=trace)
    out = np.empty((B, C, H, W), np.float32)
    for core in range(N_CORES):
        b_i, half = core // 2, core % 2
        r = np.asarray(res.results[core]["out"]).reshape(256, 64, W)
        out[b_i, :, half * 64:(half + 1) * 64, :] = r
    return out, res


def kernel(**inputs) -> np.ndarray:
    out, _ = _run(inputs, trace=False)
    return out


# revision 22
# speedup vs baseline: 1.0825x; 1.0825x over previous
"""CenterPooling (CornerNet) Trainium2 kernel — 8 NeuronCores.

Sharding: 8 cores = 4 batches x 2 H-halves.  Each core gets a host-padded
input slab [256, 70, 130] (3 halo rows each side, zero W-pad columns).

Key algebraic simplifications:
 - cummax(reverse) then cummax(forward) along an axis == global max along
   that axis, broadcast.  So the up branch only needs per-row maxes over W
   ([C, H]) and the down branch per-column maxes over H ([C, W]).
 - BN (eval mode) folds into conv weights/bias on the host; BN scale > 0 so
   max-reduction commutes with the affine+ReLU epilogue.
 - The merge conv's input is rank-structured: updown[c,h,w] = u[c,h] + d[c,w],
   so the 3x3 merge conv SEPARATES into tiny 1-D convs: an h-conv of u
   ([C, H] -> A(o,h), with 3 w-boundary classes of kx-summed weights) plus a
   w-conv of d ([C, W] -> B(o,w), with h-boundary corrections -Bk0/-Bk2 at
   the global top/bottom rows, applied data-driven via per-row selector
   vectors).  This removes the 18 big matmuls per merge block entirely.
 - Down-branch col-max needs a cross-half combine: pairwise AllReduce-max of
   a tiny [256, 128] tile.
 - H-pad semantics at the global top/bottom are handled data-driven (SPMD
   uniform program): a validity mask zeroes invalid u rows, and the per-row
   ACT bias adds -1e30 to out-of-range relu1 rows so ReLU clamps them to
   the zero-pad value.

Convs run as 9 shifted bf16 matmuls per ci-tile accumulating into fp32 PSUM
(N=512 moving operand = 4 output rows x 128 cols).
"""

import sys

sys.path.insert(0, "/opt/trn_rl_repo")

import numpy as np
import ml_dtypes

import concourse.bacc as bacc
import concourse.tile as tile
import concourse.bass as bass
from concourse import mybir, bass_utils

BF16 = mybir.dt.bfloat16
F32 = mybir.dt.float32
NP_BF16 = ml_dtypes.bfloat16

N_CORES = 8
B, CIN, C, H, W = 4, 256, 256, 128, 128
G = 3            # halo rows on each side of the 64 owned rows
HS = 64 + 2 * G  # 70 slab rows
WP = W + 2       # 130 (zero-pad col on each side)
EPS = 1e-5
NEG = -1e30

RELU = mybir.ActivationFunctionType.Relu
AX_X = mybir.AxisListType.X
ALU = mybir.AluOpType

_CACHE: dict = {}


def _mm_group(nc, ps_ap, mms):
    n = len(mms)
    for k, (lhsT, rhs) in enumerate(mms):
        nc.tensor.matmul(ps_ap, lhsT, rhs, start=(k == 0), stop=(k == n - 1))


def _conv3_mms(wtile, src, s, nr, cot):
    """The 18 (ci,ky,kx) matmuls of a 3x3 conv block: output rows s..s+nr-1."""
    mms = []
    for cit in range(2):
        for ky in range(3):
            for kx in range(3):
                j = ((ky * 3 + kx) * 2 + cit) * 2 + cot
                mms.append((wtile[:, j, :], src[cit][:, s + ky - 1:s + ky - 1 + nr, kx:kx + W]))
    return mms


def _build():
    nc = bacc.Bacc("TRN2", target_bir_lowering=False, debug=False,
                   num_devices=N_CORES)

    x_d = nc.dram_tensor("x", [2, 128, HS, WP], BF16, kind="ExternalInput")
    wup_d = nc.dram_tensor("wup", [128, 36, 128], BF16, kind="ExternalInput")
    wdn_d = nc.dram_tensor("wdn", [128, 36, 128], BF16, kind="ExternalInput")
    wc2_d = nc.dram_tensor("wc2", [128, 36, 128], BF16, kind="ExternalInput")
    wc1_d = nc.dram_tensor("wc1", [128, 4, 128], BF16, kind="ExternalInput")
    wa_d = nc.dram_tensor("wa", [128, 36, 128], BF16, kind="ExternalInput")
    wb_d = nc.dram_tensor("wb", [128, 36, 128], BF16, kind="ExternalInput")
    bias_d = nc.dram_tensor("biases", [128, 8], F32, kind="ExternalInput")
    hv_d = nc.dram_tensor("hv", [128, HS], F32, kind="ExternalInput")
    pnegb_d = nc.dram_tensor("pnegb", [128, HS], F32, kind="ExternalInput")
    htop_d = nc.dram_tensor("htopneg", [128, HS], F32, kind="ExternalInput")
    hbot_d = nc.dram_tensor("hbotneg", [128, HS], F32, kind="ExternalInput")
    out_d = nc.dram_tensor("out", [2, 128, 64, W], F32, kind="ExternalOutput")

    with tile.TileContext(nc) as tc:
        with tc.tile_pool(name="const", bufs=1) as constp, \
             tc.tile_pool(name="acts", bufs=1) as actp, \
             tc.tile_pool(name="psum", bufs=6, space="PSUM") as psp, \
             tc.tile_pool(name="ostage", bufs=6) as osp, \
             tc.tile_pool(name="dram", bufs=1, space="DRAM") as dramp:

            wdn = constp.tile([128, 36, 128], BF16)
            for j0 in range(0, 36, 9):
                nc.sync.dma_start(wdn[:, j0:j0 + 9, :], wdn_d.ap()[:, j0:j0 + 9, :])

            xs = []
            for cit in range(2):
                xt = actp.tile([128, HS, WP], BF16, name=f"xs{cit}")
                xs.append(xt)
            # interleave cit0/cit1 chunks so both tiles' early rows land first
            for r0 in range(0, HS, 8):
                r1_ = min(r0 + 8, HS)
                for cit in range(2):
                    nc.sync.dma_start(xs[cit][:, r0:r1_, :], x_d.ap()[cit, :, r0:r1_, :])

            wup = constp.tile([128, 36, 128], BF16)
            nc.sync.dma_start(wup[:, :, :], wup_d.ap())
            wc2 = constp.tile([128, 36, 128], BF16)
            nc.sync.dma_start(wc2[:, :, :], wc2_d.ap())
            wc1 = constp.tile([128, 4, 128], BF16)
            nc.sync.dma_start(wc1[:, :, :], wc1_d.ap())
            wa = constp.tile([128, 36, 128], BF16)
            nc.sync.dma_start(wa[:, :, :], wa_d.ap())
            wb = constp.tile([128, 36, 128], BF16)
            nc.sync.dma_start(wb[:, :, :], wb_d.ap())
            biases = constp.tile([128, 8], F32)
            nc.sync.dma_start(biases[:, :], bias_d.ap())
            hv = constp.tile([128, HS], F32)
            nc.sync.dma_start(hv[:, :], hv_d.ap())
            pnegb = constp.tile([128, HS], F32)
            nc.sync.dma_start(pnegb[:, :], pnegb_d.ap())
            htopneg = constp.tile([128, HS], F32)
            nc.sync.dma_start(htopneg[:, :], htop_d.ap())
            hbotneg = constp.tile([128, HS], F32)
            nc.sync.dma_start(hbotneg[:, :], hbot_d.ap())

            r1 = []
            for cit in range(2):
                t2 = actp.tile([128, HS, WP], BF16, name=f"r1{cit}")
                nc.vector.memset(t2[:, :, 0], 0.0)
                nc.vector.memset(t2[:, :, WP - 1], 0.0)
                r1.append(t2)

            uraw, ufin, dacc, dmax, dfin = [], [], [], [], []
            for cot in range(2):
                t = actp.tile([128, HS], F32, name=f"uraw{cot}")
                nc.vector.memset(t[:, :], 0.0)
                uraw.append(t)
                ufin.append(actp.tile([128, HS], F32, name=f"ufin{cot}"))
                t = actp.tile([128, W], F32, name=f"dacc{cot}")
                nc.vector.memset(t[:, :], -3e38)
                dacc.append(t)
                dmax.append(actp.tile([128, W], F32, name=f"dmax{cot}"))
                dfin.append(actp.tile([128, W], F32, name=f"dfin{cot}"))

            # ---- down branch: conv over the 64 owned rows, col-max over H ----
            for i in range(16):
                s = G + 4 * i
                for cot in range(2):
                    ps = psp.tile([128, 4, 128], F32, tag="ps", name="ps_dn", bufs=3)
                    _mm_group(nc, ps[:, :, :], _conv3_mms(wdn, xs, s, 4, cot))
                    for rr in range(4):
                        nc.vector.tensor_max(dacc[cot][:, :], dacc[cot][:, :], ps[:, rr, :])

            # pairwise (same-batch) AllReduce-max to get the global col-max
            cc_in = dramp.tile([256, W], F32)
            cc_out = dramp.tile([256, W], F32)
            for cot in range(2):
                nc.sync.dma_start(cc_in[cot * 128:(cot + 1) * 128, :], dacc[cot][:, :])
            nc.gpsimd.collective_compute(
                "AllReduce", ALU.max,
                replica_groups=[[0, 1], [2, 3], [4, 5], [6, 7]],
                ins=[cc_in.opt()], outs=[cc_out.opt()])
            for cot in range(2):
                nc.sync.dma_start(dmax[cot][:, :], cc_out[cot * 128:(cot + 1) * 128, :])
                nc.scalar.activation(dfin[cot][:, :], dmax[cot][:, :], RELU,
                                     bias=biases[:, 2 + cot:3 + cot], scale=1.0)

            # ---- up branch: conv over rows [1, 69), row-max over W ----
            for i in range(17):
                s = 1 + 4 * i
                for cot in range(2):
                    ps = psp.tile([128, 4, 128], F32, tag="ps", name="ps_up", bufs=3)
                    _mm_group(nc, ps[:, :, :], _conv3_mms(wup, xs, s, 4, cot))
                    nc.vector.reduce_max(uraw[cot][:, s:s + 4], ps[:, :, :], axis=AX_X)
            for cot in range(2):
                nc.scalar.activation(ufin[cot][:, :], uraw[cot][:, :], RELU,
                                     bias=biases[:, cot:cot + 1], scale=1.0)

            # ---- separable merge conv pieces ----
            # umask = u * hvalid (bf16), dpad = d with zero W-pad cols (bf16)
            umask, dpad = [], []
            for cit in range(2):
                t = actp.tile([128, HS], BF16, name=f"umask{cit}")
                nc.vector.tensor_mul(t[:, :], ufin[cit][:, :], hv[:, :])
                umask.append(t)
                t = actp.tile([128, WP], BF16, name=f"dpad{cit}")
                nc.vector.memset(t[:, :], 0.0)
                nc.vector.tensor_copy(t[:, 1:W + 1], dfin[cit][:, :])
                dpad.append(t)

            # A_cls(o,h): 1-D h-conv of umask with kx-summed merge weights.
            # cls 0=M (interior w), 1=L (w=0), 2=R (w=127).  Rows [2, 68).
            NA = 64 + 2
            asb = [[None, None, None], [None, None, None]]
            for cls in range(3):
                for cot in range(2):
                    psa = psp.tile([128, NA], F32, tag="psa", name="ps_a", bufs=2)
                    mms = []
                    for cit in range(2):
                        for ky in range(3):
                            j = ((cls * 3 + ky) * 2 + cit) * 2 + cot
                            mms.append((wa[:, j, :], umask[cit][:, 1 + ky:1 + ky + NA]))
                    _mm_group(nc, psa[:, :], mms)
                    t = actp.tile([128, NA], F32, name=f"asb{cls}{cot}")
                    nc.scalar.copy(t[:, :], psa[:, :])
                    asb[cot][cls] = t
            # afull = A_M + bias_pc1 + pneg (ACT bias per relu1 row);
            # afdL/afdR = A_L - A_M / A_R - A_M (w-edge fixups, pre-ReLU).
            afull, afdl, afdr = [], [], []
            for cot in range(2):
                t = actp.tile([128, HS], F32, name=f"afull{cot}")
                nc.vector.scalar_tensor_tensor(
                    t[:, 2:2 + NA], asb[cot][0][:, :], biases[:, 4 + cot:5 + cot],
                    pnegb[:, 2:2 + NA], op0=ALU.add, op1=ALU.add)
                afull.append(t)
                t = actp.tile([128, HS], F32, name=f"afdl{cot}")
                nc.vector.tensor_sub(t[:, 2:2 + NA], asb[cot][1][:, :], asb[cot][0][:, :])
                afdl.append(t)
                t = actp.tile([128, HS], F32, name=f"afdr{cot}")
                nc.vector.tensor_sub(t[:, 2:2 + NA], asb[cot][2][:, :], asb[cot][0][:, :])
                afdr.append(t)

            # B_var(o,w): 1-D w-conv of dpad with ky-summed merge weights.
            # var 0=M (all ky), 1=ky0 only, 2=ky2 only (boundary corrections).
            bt = [[None, None, None], [None, None, None]]
            for var in range(3):
                for cot in range(2):
                    psb = psp.tile([128, 128], F32, tag="psa", name="ps_b", bufs=2)
                    mms = []
                    for cit in range(2):
                        for kx in range(3):
                            j = ((var * 3 + kx) * 2 + cit) * 2 + cot
                            mms.append((wb[:, j, :], dpad[cit][:, kx:kx + W]))
                    _mm_group(nc, psb[:, :], mms)
                    t = actp.tile([128, 128], F32, name=f"bt{var}{cot}")
                    nc.scalar.copy(t[:, :], psb[:, :])
                    bt[cot][var] = t

            # ---- relu1 = relu(c1(x) + A + B + bias), assembled per block ----
            blocks = [(2 + 4 * i, 4) for i in range(16)] + [(66, 2)]
            for (s, nr) in blocks:
                for cot in range(2):
                    ps = psp.tile([128, nr, 128], F32, tag="ps", name="ps_p", bufs=3)
                    mms = []
                    for cit in range(2):
                        mms.append((wc1[:, cit * 2 + cot, :], xs[cit][:, s:s + nr, 1:W + 1]))
                    _mm_group(nc, ps[:, :, :], mms)
                    for r in range(nr):
                        sr = s + r
                        nc.vector.tensor_add(ps[:, r, :], ps[:, r, :], bt[cot][0][:, :])
                        # global top/bottom boundary corrections live at fixed
                        # slab rows (G and HS-G-1); the selector data zeroes
                        # them on the half where they don't apply.
                        if sr == G:
                            nc.vector.scalar_tensor_tensor(
                                ps[:, r, :], bt[cot][1][:, :], htopneg[:, sr:sr + 1],
                                ps[:, r, :], op0=ALU.mult, op1=ALU.add)
                        if sr == HS - G - 1:
                            nc.vector.scalar_tensor_tensor(
                                ps[:, r, :], bt[cot][2][:, :], hbotneg[:, sr:sr + 1],
                                ps[:, r, :], op0=ALU.mult, op1=ALU.add)
                    nc.vector.tensor_add(ps[:, :, 0], ps[:, :, 0], afdl[cot][:, s:s + nr])
                    nc.vector.tensor_add(ps[:, :, W - 1], ps[:, :, W - 1], afdr[cot][:, s:s + nr])
                    for r in range(nr):
                        sr = s + r
                        nc.scalar.activation(r1[cot][:, sr, 1:W + 1], ps[:, r, :], RELU,
                                             bias=afull[cot][:, sr:sr + 1], scale=1.0)

            # ---- output conv block ----
            for i in range(16):
                s = G + 4 * i
                for cot in range(2):
                    ps = psp.tile([128, 4, 128], F32, tag="ps2", name="ps_c2", bufs=3)
                    _mm_group(nc, ps[:, :, :], _conv3_mms(wc2, r1, s, 4, cot))
                    ot = osp.tile([128, 4, 128], F32, name="ot")
                    nc.scalar.activation(ot[:, :, :], ps[:, :, :], RELU,
                                         bias=biases[:, 6 + cot:7 + cot], scale=1.0)
                    if i >= 14:
                        # split the tail stores across rings so the last
                        # store's serial latency is halved
                        nc.sync.dma_start(out_d.ap()[cot, :, s - G:s - G + 2, :], ot[:, 0:2, :])
                        nc.sync.dma_start(out_d.ap()[cot, :, s - G + 2:s - G + 4, :], ot[:, 2:4, :])
                    else:
                        nc.sync.dma_start(out_d.ap()[cot, :, s - G:s - G + 4, :], ot[:, :, :])

    nc.compile()
    return nc


def _pack3(w):
    # [256o, 256i, 3, 3] -> [128ci, j, 128co], j = ((ky*3+kx)*2+cit)*2+cot
    a = w.reshape(2, 128, 2, 128, 3, 3).transpose(3, 4, 5, 2, 0, 1)
    return np.ascontiguousarray(a.reshape(128, 36, 128)).astype(NP_BF16)


def _pack1(w):
    # [256o, 256i, 1, 1] -> [128ci, j, 128co], j = cit*2+cot
    a = w[:, :, 0, 0].reshape(2, 128, 2, 128).transpose(3, 2, 0, 1)
    return np.ascontiguousarray(a.reshape(128, 4, 128)).astype(NP_BF16)


def _pack_sep(wk3):
    # wk3: [256o, 256i, 3] (kx- or ky-summed variants stacked on axis -1 by
    # caller as a dict) -> packs a [3var/cls, 3k, 256, 256] stack into
    # [128ci, j, 128co], j = ((v*3+k)*2+cit)*2+cot
    a = wk3.reshape(3, 3, 2, 128, 2, 128).transpose(5, 0, 1, 4, 2, 3)
    # dims now [ci, v, k, cit, cot, co]
    return np.ascontiguousarray(a.reshape(128, 36, 128)).astype(NP_BF16)


def _prep_in_maps(inputs):
    x = np.asarray(inputs["x"], dtype=np.float32)

    fw, fb = {}, {}
    for n in ["up", "down", "p", "c1", "c2"]:
        g = np.asarray(inputs[f"g_{n}"], np.float32)
        v = np.asarray(inputs[f"v_{n}"], np.float32)
        m = np.asarray(inputs[f"m_{n}"], np.float32)
        b = np.asarray(inputs[f"b_{n}"], np.float32)
        w = np.asarray(inputs[f"w_{n}"], np.float32)
        s = g / np.sqrt(v + EPS)
        fw[n] = w * s[:, None, None, None]
        fb[n] = b - m * s

    wp = fw["p"]
    wa_stack = np.stack([
        np.stack([wp[:, :, ky, :].sum(-1) for ky in range(3)]),            # M
        np.stack([wp[:, :, ky, 1:].sum(-1) for ky in range(3)]),           # L (w=0)
        np.stack([wp[:, :, ky, :2].sum(-1) for ky in range(3)]),           # R (w=127)
    ])
    wb_stack = np.stack([
        np.stack([wp[:, :, :, kx].sum(-1) for kx in range(3)]),            # M
        np.stack([wp[:, :, 0, kx] for kx in range(3)]),                    # ky=0
        np.stack([wp[:, :, 2, kx] for kx in range(3)]),                    # ky=2
    ])
    consts = {
        "wup": _pack3(fw["up"]),
        "wdn": _pack3(fw["down"]),
        "wc2": _pack3(fw["c2"]),
        "wc1": _pack1(fw["c1"]),
        "wa": _pack_sep(wa_stack),
        "wb": _pack_sep(wb_stack),
    }
    bias_np = np.zeros((128, 8), np.float32)
    for k, arr in enumerate([fb["up"], fb["down"], fb["p"] + fb["c1"], fb["c2"]]):
        m2 = arr.reshape(2, 128)
        bias_np[:, 2 * k] = m2[0]
        bias_np[:, 2 * k + 1] = m2[1]
    consts["biases"] = bias_np

    def _bcast(row):
        return np.ascontiguousarray(
            np.broadcast_to(row.astype(np.float32)[None, :], (128, HS)))

    in_maps = []
    for core in range(N_CORES):
        b_i, half = core // 2, core % 2
        slab = np.zeros((256, HS, WP), np.float32)
        if half == 0:
            slab[:, G:, 1:W + 1] = x[b_i][:, 0:HS - G, :]
            hv_row = (np.arange(HS) >= G)
            top_s, bot_s = G, None            # slab row of global row 0
        else:
            slab[:, :HS - G, 1:W + 1] = x[b_i][:, H - (HS - G):H, :]
            hv_row = (np.arange(HS) <= HS - G - 1)
            top_s, bot_s = None, HS - G - 1   # slab row of global row H-1
        xsl = np.ascontiguousarray(slab.reshape(2, 128, HS, WP)).astype(NP_BF16)
        pneg_row = np.where(hv_row, 0.0, NEG)
        htop_row = np.zeros(HS)
        if top_s is not None:
            htop_row[top_s] = -1.0
        hbot_row = np.zeros(HS)
        if bot_s is not None:
            hbot_row[bot_s] = -1.0
        in_maps.append({
            "x": xsl, "hv": _bcast(hv_row), "pnegb": _bcast(pneg_row),
            "htopneg": _bcast(htop_row), "hbotneg": _bcast(hbot_row), **consts})
    return in_maps


def _run(inputs, trace=False):
    # Build a fresh Bass program per call: re-executing an already-loaded
    # NEFF on these cores intermittently trips NRT_EXEC_UNIT_UNRECOVERABLE,
    # while a fresh build+load is reliable (neuronxcc cache keeps it fast).
    nc = _build()
    in_maps = _prep_in_maps(inputs)
    res = bass_utils.run_bass_kernel_spmd(
        nc, in_maps, core_ids=list(range(N_CORES)), trace=trace)
    out = np.empty((B, C, H, W), np.float32)
    for core in range(N_CORES):
        b_i, half = core // 2, core % 2
        r = np.asarray(res.results[core]["out"]).reshape(256, 64, W)
        out[b_i, :, half * 64:(half + 1) * 64, :] = r
    return out, res


def kernel(**inputs) -> np.ndarray:
    out, _ = _run(inputs, trace=False)
    return out
